# revision 1
# baseline (speedup 1.0000x reference)
"""Trainium2 Bass kernel: 12-head attention block (qkv proj -> softmax attn -> fc).

Reference semantics (B=32, S=577, D=768, H=12, Dh=64):
    qkv = x @ w_qkv + b_qkv
    q, k, v = split(qkv); attn = softmax(q k^T / 8) v
    out = attn @ w_fc + b_fc

Sharding: data-parallel over batch across 8 NeuronCores (4 images per core),
weights replicated, no collectives. Compute in bf16 with fp32 PSUM accumulation.

Layout strategy per core (all matmuls contract over the partition dim):
  - xT [768, 577] built from x via PE transposes (fp32 transpose mode).
  - qkT [1536, 577] = w_qkv[:, :1536]^T . xT  (w_qkv stationary in natural layout).
  - v   [577, 768]  = xT^T . w_qkv[:, 1536:]  (natural layout, per-head ones column
    appended so attention row-sums fall out of the attn@v matmul for free).
  - scoresT[sk, sq] = kT_h^T . qT_h per head; exp on ScalarE (scale=1/8 folded in;
    no max subtraction -- scores are O(1) here, exp is safe in fp32).
  - attn_outT[65, sq] = (v_h|1)^T . expT ; row 64 = softmax denominators.
  - normalize via reciprocal + K=1 broadcast matmul, writing attn_T [768, 577].
  - fc: out[s, :] = attn_T_k^T . w_fc_k (natural w_fc), + b_fc broadcast.
"""

import os
import sys

import numpy as np

for _p in ("/opt/trn_rl_repo", "/root/.axon_site/_ro/trn_rl_repo"):
    if os.path.isdir(_p) and _p not in sys.path:
        sys.path.insert(0, _p)

import concourse.bass as bass  # noqa: E402
import concourse.tile as tile  # noqa: E402
from concourse import bacc, mybir  # noqa: E402
from concourse.bass_utils import run_bass_kernel_spmd  # noqa: E402
from concourse.masks import make_identity  # noqa: E402

F32 = mybir.dt.float32
BF16 = mybir.dt.bfloat16

B, S, D = 32, 577, 768
H, DH = 12, 64
NCORES = 8
NB = B // NCORES  # 4 batch images per core
SCALE = DH**-0.5  # 0.125
NKT = D // 128  # 6 contraction tiles of 128
S_TILES = [(0, 128), (128, 128), (256, 128), (384, 128), (512, 65)]
CH_S = [(0, 512), (512, 65)]  # 577 split at PSUM-bank boundary
CH_D = [(0, 512), (512, 256)]  # 768 split at PSUM-bank boundary
EXP = mybir.ActivationFunctionType.Exp
IDENT = mybir.ActivationFunctionType.Identity


def build_nc():
    nc = bacc.Bacc(None)
    x_ext = nc.declare_dram_parameter("x", [NB, S, D], F32, isOutput=False)
    wqkv_ext = nc.declare_dram_parameter("w_qkv", [D, 3 * D], F32, isOutput=False)
    bqkv_ext = nc.declare_dram_parameter("b_qkv", [3 * D], F32, isOutput=False)
    wfc_ext = nc.declare_dram_parameter("w_fc", [D, D], F32, isOutput=False)
    bfc_ext = nc.declare_dram_parameter("b_fc", [D], F32, isOutput=False)
    out_ext = nc.declare_dram_parameter("out", [NB, S, D], F32, isOutput=True)

    with tile.TileContext(nc) as tc:
        with (
            tc.tile_pool(name="const", bufs=1) as cpool,
            tc.tile_pool(name="wtmp", bufs=1) as wtmp_pool,
            tc.tile_pool(name="x", bufs=2) as x_pool,
            tc.tile_pool(name="xT", bufs=2) as xT_pool,
            tc.tile_pool(name="qkT", bufs=2) as qkT_pool,
            tc.tile_pool(name="v", bufs=2) as v_pool,
            tc.tile_pool(name="expT", bufs=2) as expT_pool,
            tc.tile_pool(name="attnT", bufs=2) as attnT_pool,
            tc.tile_pool(name="small", bufs=3) as small_pool,
            tc.tile_pool(name="osb", bufs=3) as osb_pool,
            tc.tile_pool(name="ps", bufs=4, space="PSUM") as ps,
        ):
            # ---- constants / weights (once) ----
            identity = cpool.tile([128, 128], F32)
            make_identity(nc, identity[:])
            ones = cpool.tile([1, 128], F32)
            nc.vector.memset(ones[:], 1.0)

            b_qk = cpool.tile([128, H], F32)  # per-partition bias for qkT tiles
            nc.sync.dma_start(
                b_qk[:], bqkv_ext[0 : 2 * D].rearrange("(m p) -> p m", p=128)
            )
            brow_v = cpool.tile([1, D], F32)
            nc.sync.dma_start(brow_v[:], bqkv_ext[None, 2 * D : 3 * D])
            brow_fc = cpool.tile([1, D], F32)
            nc.sync.dma_start(brow_fc[:], bfc_ext[None, :])

            # broadcast biases to all 128 partitions via K=1 matmul
            b_v_bc = cpool.tile([128, D], F32)
            b_fc_bc = cpool.tile([128, D], F32)
            for row, bc in ((brow_v, b_v_bc), (brow_fc, b_fc_bc)):
                pb = ps.tile([128, D], F32, tag="ps")
                for c0, cl in CH_D:
                    nc.tensor.matmul(
                        pb[:, c0 : c0 + cl],
                        lhsT=ones[0:1, 0:128],
                        rhs=row[0:1, c0 : c0 + cl],
                        start=True,
                        stop=True,
                    )
                nc.vector.tensor_copy(bc[:], pb[:])

            # weights -> SBUF bf16 (w_qkv_all block k at free [k*2304, (k+1)*2304))
            w_qkv_all = cpool.tile([128, NKT * 3 * D], BF16)
            w_fc_all = cpool.tile([128, NKT * D], BF16)
            for k in range(NKT):
                wt = wtmp_pool.tile([128, 3 * D], F32, tag="wtmp")
                nc.sync.dma_start(wt[:], wqkv_ext[k * 128 : (k + 1) * 128, :])
                nc.vector.tensor_copy(
                    w_qkv_all[:, k * 3 * D : (k + 1) * 3 * D], wt[:]
                )
                wf = wtmp_pool.tile([128, D], F32, tag="wfc_tmp")
                nc.sync.dma_start(wf[:], wfc_ext[k * 128 : (k + 1) * 128, :])
                nc.vector.tensor_copy(w_fc_all[:, k * D : (k + 1) * D], wf[:])

            # ---- per batch image ----
            for b in range(NB):
                # x [577, 768] fp32, 5 row tiles packed in the free dim
                x_all = x_pool.tile([128, 5 * D], F32, tag="x")
                for si, (s0, psl) in enumerate(S_TILES):
                    nc.sync.dma_start(
                        x_all[0:psl, si * D : (si + 1) * D],
                        x_ext[b, s0 : s0 + psl, :],
                    )

                # xT [768, 577] bf16 via PE transposes
                xT_all = xT_pool.tile([128, NKT * S], BF16, tag="xT")
                for dk in range(NKT):
                    px = ps.tile([128, S], F32, tag="ps")
                    for si, (s0, psl) in enumerate(S_TILES):
                        nc.tensor.transpose(
                            px[:, s0 : s0 + psl],
                            x_all[0:psl, si * D + dk * 128 : si * D + (dk + 1) * 128],
                            identity[0:psl, 0:psl],
                        )
                    nc.vector.tensor_copy(xT_all[:, dk * S : (dk + 1) * S], px[:])

                # qkT [1536, 577]: tile m holds rows [m*128, (m+1)*128) = 2 heads
                qkT_all = qkT_pool.tile([128, 2 * NKT * S], BF16, tag="qkT")
                for m in range(2 * NKT):
                    pqk = ps.tile([128, S], F32, tag="ps")
                    for k in range(NKT):
                        for c0, cl in CH_S:
                            nc.tensor.matmul(
                                pqk[:, c0 : c0 + cl],
                                lhsT=w_qkv_all[
                                    :, k * 3 * D + m * 128 : k * 3 * D + (m + 1) * 128
                                ],
                                rhs=xT_all[:, k * S + c0 : k * S + c0 + cl],
                                start=(k == 0),
                                stop=(k == NKT - 1),
                            )
                    nc.scalar.activation(
                        qkT_all[:, m * S : (m + 1) * S],
                        pqk[:],
                        IDENT,
                        bias=b_qk[:, m : m + 1],
                    )

                # v natural [577, 768] + per-head ones column (65 floats per head)
                v_all = v_pool.tile([128, 5 * H * (DH + 1)], BF16, tag="v")
                v4 = v_all[:].rearrange("p (s h e) -> p s h e", s=5, h=H)
                nc.vector.memset(v4[:, :, :, DH : DH + 1], 1.0)
                for si, (s0, psl) in enumerate(S_TILES):
                    pv = ps.tile([128, D], F32, tag="ps")
                    for k in range(NKT):
                        for c0, cl in CH_D:
                            nc.tensor.matmul(
                                pv[0:psl, c0 : c0 + cl],
                                lhsT=xT_all[:, k * S + s0 : k * S + s0 + psl],
                                rhs=w_qkv_all[
                                    :, k * 3 * D + 2 * D + c0 : k * 3 * D + 2 * D + c0 + cl
                                ],
                                start=(k == 0),
                                stop=(k == NKT - 1),
                            )
                    nc.vector.tensor_add(
                        v4[0:psl, si, :, 0:DH],
                        pv[0:psl, :].rearrange("p (h e) -> p h e", h=H),
                        b_v_bc[0:psl, :].rearrange("p (h e) -> p h e", h=H),
                    )

                # attention per head; attn_T [768, 577] assembled transposed
                attnT_all = attnT_pool.tile([128, NKT * S], BF16, tag="attnT")
                for h in range(H):
                    hoff = (h % 2) * 64
                    qm, km = h // 2, NKT + h // 2
                    # scoresT [sk, sq] (per sk tile) -> exp -> expT bf16
                    expT_all = expT_pool.tile([128, 5 * S], BF16, tag="expT")
                    for si, (s0, psl) in enumerate(S_TILES):
                        psc = ps.tile([128, S], F32, tag="ps")
                        for c0, cl in CH_S:
                            nc.tensor.matmul(
                                psc[0:psl, c0 : c0 + cl],
                                lhsT=qkT_all[
                                    hoff : hoff + 64, km * S + s0 : km * S + s0 + psl
                                ],
                                rhs=qkT_all[hoff : hoff + 64, qm * S + c0 : qm * S + c0 + cl],
                                start=True,
                                stop=True,
                            )
                        nc.scalar.activation(
                            expT_all[0:psl, si * S : (si + 1) * S],
                            psc[0:psl, :],
                            EXP,
                            scale=float(SCALE),
                        )
                    # attn_outT [65, 577]: rows 0:64 = unnormalized out^T, row 64 = sums
                    po = ps.tile([65, S], F32, tag="ps")
                    for si, (s0, psl) in enumerate(S_TILES):
                        for c0, cl in CH_S:
                            nc.tensor.matmul(
                                po[:, c0 : c0 + cl],
                                lhsT=v_all[
                                    0:psl,
                                    si * H * (DH + 1)
                                    + h * (DH + 1) : si * H * (DH + 1)
                                    + (h + 1) * (DH + 1),
                                ],
                                rhs=expT_all[0:psl, si * S + c0 : si * S + c0 + cl],
                                start=(si == 0),
                                stop=(si == 4),
                            )
                    rinv = small_pool.tile([1, S], F32, tag="rinv")
                    nc.vector.reciprocal(rinv[:], po[64:65, :])
                    pr = ps.tile([64, S], F32, tag="ps")
                    for c0, cl in CH_S:
                        nc.tensor.matmul(
                            pr[:, c0 : c0 + cl],
                            lhsT=ones[0:1, 0:64],
                            rhs=rinv[0:1, c0 : c0 + cl],
                            start=True,
                            stop=True,
                        )
                    rbc = small_pool.tile([64, S], F32, tag="rbc")
                    nc.scalar.copy(rbc[:], pr[:])
                    nc.vector.tensor_mul(
                        attnT_all[hoff : hoff + 64, (h // 2) * S : (h // 2 + 1) * S],
                        po[0:64, :],
                        rbc[:],
                    )

                # fc: out rows [s0, s0+psl)
                for si, (s0, psl) in enumerate(S_TILES):
                    pf = ps.tile([128, D], F32, tag="ps")
                    for k in range(NKT):
                        for c0, cl in CH_D:
                            nc.tensor.matmul(
                                pf[0:psl, c0 : c0 + cl],
                                lhsT=attnT_all[:, k * S + s0 : k * S + s0 + psl],
                                rhs=w_fc_all[:, k * D + c0 : k * D + c0 + cl],
                                start=(k == 0),
                                stop=(k == NKT - 1),
                            )
                    osb = osb_pool.tile([128, D], F32, tag="osb")
                    nc.vector.tensor_add(osb[0:psl, :], pf[0:psl, :], b_fc_bc[0:psl, :])
                    nc.sync.dma_start(out_ext[b, s0 : s0 + psl, :], osb[0:psl, :])

    nc.compile()
    return nc


_NC_CACHE = None


def _get_nc():
    global _NC_CACHE
    if _NC_CACHE is None:
        _NC_CACHE = build_nc()
    return _NC_CACHE


def kernel(x, w_qkv, b_qkv, w_fc, b_fc, _collect=None):
    nc = _get_nc()
    x = np.ascontiguousarray(np.asarray(x, dtype=np.float32))
    w_qkv = np.ascontiguousarray(np.asarray(w_qkv, dtype=np.float32))
    b_qkv = np.ascontiguousarray(np.asarray(b_qkv, dtype=np.float32))
    w_fc = np.ascontiguousarray(np.asarray(w_fc, dtype=np.float32))
    b_fc = np.ascontiguousarray(np.asarray(b_fc, dtype=np.float32))
    in_maps = [
        {
            "x": x[i * NB : (i + 1) * NB],
            "w_qkv": w_qkv,
            "b_qkv": b_qkv,
            "w_fc": w_fc,
            "b_fc": b_fc,
        }
        for i in range(NCORES)
    ]
    kwargs = dict(_collect) if _collect else {}
    res = run_bass_kernel_spmd(nc, in_maps, core_ids=list(range(NCORES)), **kwargs)
    out = np.concatenate([res.results[i]["out"] for i in range(NCORES)], axis=0)
    if _collect is not None and isinstance(_collect, dict):
        _collect["result"] = res
    return out.astype(np.float32)


if __name__ == "__main__":
    xs = np.random.randn(B, S, D).astype(np.float32)
    lim = 1.0 / np.sqrt(D)
    rng = np.random.default_rng(0)
    wq = rng.uniform(-lim, lim, (D, 3 * D)).astype(np.float32)
    bq = rng.uniform(-lim, lim, (3 * D,)).astype(np.float32)
    wf = rng.uniform(-lim, lim, (D, D)).astype(np.float32)
    bf = rng.uniform(-lim, lim, (D,)).astype(np.float32)
    o = kernel(xs, wq, bq, wf, bf)
    print("out", o.shape, o.dtype)


# revision 8
# speedup vs baseline: 1.5179x; 1.5179x over previous
"""Trainium2 Bass kernel: 12-head attention block (qkv proj -> softmax attn -> fc).

Reference semantics (B=32, S=577, D=768, H=12, Dh=64):
    qkv = x @ w_qkv + b_qkv
    q, k, v = split(qkv); attn = softmax(q k^T / 8) v
    out = attn @ w_fc + b_fc

Sharding: data-parallel over batch across 8 NeuronCores (4 images per core),
weights replicated, no collectives. Compute in bf16 with fp32 PSUM accumulation.

Layout strategy per core (all matmuls contract over the partition dim):
  - xT [768, 577] built from x via PE transposes (fp32 transpose mode).
  - qkT [1536, 577] = w_qkv[:, :1536]^T . xT  (w_qkv stationary in natural layout).
  - v   [577, 768]  = xT^T . w_qkv[:, 1536:]  (natural layout, per-head ones column
    appended so attention row-sums fall out of the attn@v matmul for free).
  - scoresT[sk, sq] = kT_h^T . qT_h per head; exp on ScalarE (scale=1/8 folded in;
    no max subtraction -- scores are O(1) here, exp is safe in fp32).
  - attn_outT[65, sq] = (v_h|1)^T . expT ; row 64 = softmax denominators.
  - normalize via reciprocal + K=1 broadcast matmul, writing attn_T [768, 577].
  - fc: out[s, :] = attn_T_k^T . w_fc_k (natural w_fc), + b_fc broadcast.
"""

import os
import sys

import numpy as np

for _p in ("/opt/trn_rl_repo", "/root/.axon_site/_ro/trn_rl_repo"):
    if os.path.isdir(_p) and _p not in sys.path:
        sys.path.insert(0, _p)

import concourse.bass as bass  # noqa: E402
import concourse.tile as tile  # noqa: E402
from concourse import bacc, mybir  # noqa: E402
from concourse.bass_utils import run_bass_kernel_spmd  # noqa: E402
from concourse.masks import make_identity  # noqa: E402

F32 = mybir.dt.float32
BF16 = mybir.dt.bfloat16

B, S, D = 32, 577, 768
H, DH = 12, 64
NCORES = 8
NB = B // NCORES  # 4 batch images per core
SCALE = DH**-0.5  # 0.125
NKT = D // 128  # 6 contraction tiles of 128
S_TILES = [(0, 128), (128, 128), (256, 128), (384, 128), (512, 65)]
CH_S = [(0, 512), (512, 65)]  # 577 split at PSUM-bank boundary
CH_D = [(0, 512), (512, 256)]  # 768 split at PSUM-bank boundary
EXP = mybir.ActivationFunctionType.Exp
IDENT = mybir.ActivationFunctionType.Identity


def build_nc():
    nc = bacc.Bacc(None)
    x_ext = nc.declare_dram_parameter("x", [NB, S, D], F32, isOutput=False)
    wqkv_ext = nc.declare_dram_parameter("w_qkv", [D, 3 * D], F32, isOutput=False)
    bqkv_ext = nc.declare_dram_parameter("b_qkv", [3 * D], F32, isOutput=False)
    wfc_ext = nc.declare_dram_parameter("w_fc", [D, D], F32, isOutput=False)
    bfc_ext = nc.declare_dram_parameter("b_fc", [D], F32, isOutput=False)
    out_ext = nc.declare_dram_parameter("out", [NB, S, D], F32, isOutput=True)

    with tile.TileContext(nc) as tc:
        with (
            tc.tile_pool(name="const", bufs=1) as cpool,
            tc.tile_pool(name="wtmp", bufs=1) as wtmp_pool,
            tc.tile_pool(name="x", bufs=1) as x_pool,
            tc.tile_pool(name="xT", bufs=2) as xT_pool,
            tc.tile_pool(name="qkT", bufs=2) as qkT_pool,
            tc.tile_pool(name="v", bufs=2) as v_pool,
            tc.tile_pool(name="expT", bufs=3) as expT_pool,
            tc.tile_pool(name="attnT", bufs=2) as attnT_pool,
            tc.tile_pool(name="small", bufs=3) as small_pool,
            tc.tile_pool(name="osb", bufs=2) as osb_pool,
            tc.tile_pool(name="ps", bufs=4, space="PSUM") as ps,
        ):
            # ---- constants / weights (once) ----
            identity = cpool.tile([128, 128], F32)
            make_identity(nc, identity[:])
            ones = cpool.tile([1, 128], F32)
            nc.vector.memset(ones[:], 1.0)

            b_qk = cpool.tile([128, H], F32)  # per-partition bias for qkT tiles
            nc.sync.dma_start(
                b_qk[:], bqkv_ext[0 : 2 * D].rearrange("(m p) -> p m", p=128)
            )
            brow_v = cpool.tile([1, D], F32)
            nc.sync.dma_start(brow_v[:], bqkv_ext[None, 2 * D : 3 * D])
            brow_fc = cpool.tile([1, D], F32)
            nc.sync.dma_start(brow_fc[:], bfc_ext[None, :])

            # broadcast biases to all 128 partitions via K=1 matmul
            b_v_bc = cpool.tile([128, D], F32)
            b_fc_bc = cpool.tile([128, D], F32)
            for row, bc in ((brow_v, b_v_bc), (brow_fc, b_fc_bc)):
                pb = ps.tile([128, D], F32, tag="ps")
                for c0, cl in CH_D:
                    nc.tensor.matmul(
                        pb[:, c0 : c0 + cl],
                        lhsT=ones[0:1, 0:128],
                        rhs=row[0:1, c0 : c0 + cl],
                        start=True,
                        stop=True,
                    )
                nc.vector.tensor_copy(bc[:], pb[:])

            # weights -> SBUF bf16 (w_qkv_all block k at free [k*2304, (k+1)*2304))
            w_qkv_all = cpool.tile([128, NKT * 3 * D], BF16)
            w_fc_all = cpool.tile([128, NKT * D], BF16)
            HW = 3 * D // 2
            for k in range(NKT):
                for j in range(2):
                    wt = wtmp_pool.tile([128, HW], F32, tag="wtmp", name="wt")
                    nc.sync.dma_start(
                        wt[:], wqkv_ext[k * 128 : (k + 1) * 128, j * HW : (j + 1) * HW]
                    )
                    nc.vector.tensor_copy(
                        w_qkv_all[:, k * 3 * D + j * HW : k * 3 * D + (j + 1) * HW],
                        wt[:],
                    )
                wf = wtmp_pool.tile([128, D], F32, tag="wtmp", name="wf")
                nc.sync.dma_start(wf[:], wfc_ext[k * 128 : (k + 1) * 128, :])
                nc.vector.tensor_copy(w_fc_all[:, k * D : (k + 1) * D], wf[:])

            # ---- per batch image ----
            for b in range(NB):
                # x [577, 768] fp32, 5 row tiles packed in the free dim
                x_all = x_pool.tile([128, 5 * D], F32, tag="x")
                for si, (s0, psl) in enumerate(S_TILES):
                    nc.sync.dma_start(
                        x_all[0:psl, si * D : (si + 1) * D],
                        x_ext[b, s0 : s0 + psl, :],
                    )

                # xT [768, 577] bf16 via PE transposes
                xT_all = xT_pool.tile([128, NKT * S], BF16, tag="xT")
                for dk in range(NKT):
                    px = ps.tile([128, S], F32, tag="ps")
                    for si, (s0, psl) in enumerate(S_TILES):
                        nc.tensor.transpose(
                            px[:, s0 : s0 + psl],
                            x_all[0:psl, si * D + dk * 128 : si * D + (dk + 1) * 128],
                            identity[0:psl, 0:psl],
                        )
                    nc.vector.tensor_copy(xT_all[:, dk * S : (dk + 1) * S], px[:])

                # qkT [1536, 577]: tile m holds rows [m*128, (m+1)*128) = 2 heads
                qkT_all = qkT_pool.tile([128, 2 * NKT * S], BF16, tag="qkT")
                for m in range(2 * NKT):
                    pqk = ps.tile([128, S], F32, tag="ps")
                    for k in range(NKT):
                        for c0, cl in CH_S:
                            nc.tensor.matmul(
                                pqk[:, c0 : c0 + cl],
                                lhsT=w_qkv_all[
                                    :, k * 3 * D + m * 128 : k * 3 * D + (m + 1) * 128
                                ],
                                rhs=xT_all[:, k * S + c0 : k * S + c0 + cl],
                                start=(k == 0),
                                stop=(k == NKT - 1),
                            )
                    nc.scalar.activation(
                        qkT_all[:, m * S : (m + 1) * S],
                        pqk[:],
                        IDENT,
                        bias=b_qk[:, m : m + 1],
                    )

                # v natural [577, 768] + per-head ones column (65 floats per head)
                v_all = v_pool.tile([128, 5 * H * (DH + 1)], BF16, tag="v")
                v4 = v_all[:].rearrange("p (s h e) -> p s h e", s=5, h=H)
                nc.vector.memset(v4[:, :, :, DH : DH + 1], 1.0)
                for si, (s0, psl) in enumerate(S_TILES):
                    pv = ps.tile([128, D], F32, tag="ps")
                    for k in range(NKT):
                        for c0, cl in CH_D:
                            nc.tensor.matmul(
                                pv[0:psl, c0 : c0 + cl],
                                lhsT=xT_all[:, k * S + s0 : k * S + s0 + psl],
                                rhs=w_qkv_all[
                                    :, k * 3 * D + 2 * D + c0 : k * 3 * D + 2 * D + c0 + cl
                                ],
                                start=(k == 0),
                                stop=(k == NKT - 1),
                            )
                    nc.vector.tensor_add(
                        v4[0:psl, si, :, 0:DH],
                        pv[0:psl, :].rearrange("p (h e) -> p h e", h=H),
                        b_v_bc[0:psl, :].rearrange("p (h e) -> p h e", h=H),
                    )

                # attention, head pairs (2p, 2p+1): even head uses PE rows 0:64,
                # odd head rows 64:128 -> score matmuls run concurrently.
                # attn_T [768, 577] assembled transposed (unnormalized, then
                # scaled in place by 1/rowsum via gpsimd partition_broadcast).
                attnT_all = attnT_pool.tile([128, NKT * S], BF16, tag="attnT")
                for p in range(H // 2):
                    heads = (2 * p, 2 * p + 1)
                    expT = {}
                    for h in heads:
                        expT[h] = expT_pool.tile([128, 5 * S], BF16, tag="expT", name=f"expT{h%2}")
                    for si, (s0, psl) in enumerate(S_TILES):
                        psc = {}
                        for h in heads:
                            psc[h] = ps.tile([128, S], F32, tag="ps", name=f"psc{h%2}")
                        for c0, cl in CH_S:
                            for h in heads:
                                hoff = (h % 2) * 64
                                qm, km = h // 2, NKT + h // 2
                                nc.tensor.matmul(
                                    psc[h][0:psl, c0 : c0 + cl],
                                    lhsT=qkT_all[
                                        hoff : hoff + 64,
                                        km * S + s0 : km * S + s0 + psl,
                                    ],
                                    rhs=qkT_all[
                                        hoff : hoff + 64, qm * S + c0 : qm * S + c0 + cl
                                    ],
                                    start=True,
                                    stop=True,
                                )
                        for h in heads:
                            nc.scalar.activation(
                                expT[h][0:psl, si * S : (si + 1) * S],
                                psc[h][0:psl, :],
                                EXP,
                                scale=float(SCALE),
                            )
                    rinv = {}
                    for h in heads:
                        hoff = (h % 2) * 64
                        # attn_outT [65, 577]: rows 0:64 = out^T unnorm, row 64 = sums
                        po = ps.tile([65, S], F32, tag="ps")
                        for si, (s0, psl) in enumerate(S_TILES):
                            for c0, cl in CH_S:
                                nc.tensor.matmul(
                                    po[:, c0 : c0 + cl],
                                    lhsT=v_all[
                                        0:psl,
                                        si * H * (DH + 1)
                                        + h * (DH + 1) : si * H * (DH + 1)
                                        + (h + 1) * (DH + 1),
                                    ],
                                    rhs=expT[h][0:psl, si * S + c0 : si * S + c0 + cl],
                                    start=(si == 0),
                                    stop=(si == 4),
                                )
                        # drain po fast: unnormalized copy + fast reciprocal
                        nc.vector.tensor_copy(
                            attnT_all[
                                hoff : hoff + 64, (h // 2) * S : (h // 2 + 1) * S
                            ],
                            po[0:64, :],
                        )
                        # stage sums to SBUF base 0: reciprocal_approx_fast
                        # reads the wrong rows on HW for a nonzero input base
                        rs = small_pool.tile([1, S], F32, tag="rs", name=f"rs{h%2}")
                        nc.vector.tensor_copy(rs[:], po[64:65, :])
                        rinv[h] = small_pool.tile([1, S], F32, tag="rinv", name=f"rinv{h%2}")
                        nc.vector.reciprocal_approx_fast(rinv[h][:], rs[:])
                    for h in heads:
                        hoff = (h % 2) * 64
                        # rbc rows live at the same partition offset as the
                        # attnT slice (walrus: SBUF tensor_tensor operands
                        # must share a start partition)
                        rbc = small_pool.tile([128, S], F32, tag="rbc")
                        # broadcast to all 128 rows: HW partition_broadcast
                        # only writes correctly when the dest starts at row 0
                        nc.gpsimd.partition_broadcast(rbc[:, :], rinv[h][0:1, :])
                        nc.vector.tensor_mul(
                            attnT_all[
                                hoff : hoff + 64, (h // 2) * S : (h // 2 + 1) * S
                            ],
                            attnT_all[
                                hoff : hoff + 64, (h // 2) * S : (h // 2 + 1) * S
                            ],
                            rbc[hoff : hoff + 64, :],
                        )

                # fc: out rows [s0, s0+psl)
                for si, (s0, psl) in enumerate(S_TILES):
                    pf = ps.tile([128, D], F32, tag="ps")
                    for k in range(NKT):
                        for c0, cl in CH_D:
                            nc.tensor.matmul(
                                pf[0:psl, c0 : c0 + cl],
                                lhsT=attnT_all[:, k * S + s0 : k * S + s0 + psl],
                                rhs=w_fc_all[:, k * D + c0 : k * D + c0 + cl],
                                start=(k == 0),
                                stop=(k == NKT - 1),
                            )
                    osb = osb_pool.tile([128, D], F32, tag="osb")
                    nc.vector.tensor_add(osb[0:psl, :], pf[0:psl, :], b_fc_bc[0:psl, :])
                    nc.sync.dma_start(out_ext[b, s0 : s0 + psl, :], osb[0:psl, :])

    nc.compile()
    return nc


_NC_CACHE = None


def _get_nc():
    global _NC_CACHE
    if _NC_CACHE is None:
        _NC_CACHE = build_nc()
    return _NC_CACHE


def kernel(x, w_qkv, b_qkv, w_fc, b_fc, _collect=None):
    nc = _get_nc()
    x = np.ascontiguousarray(np.asarray(x, dtype=np.float32))
    w_qkv = np.ascontiguousarray(np.asarray(w_qkv, dtype=np.float32))
    b_qkv = np.ascontiguousarray(np.asarray(b_qkv, dtype=np.float32))
    w_fc = np.ascontiguousarray(np.asarray(w_fc, dtype=np.float32))
    b_fc = np.ascontiguousarray(np.asarray(b_fc, dtype=np.float32))
    in_maps = [
        {
            "x": x[i * NB : (i + 1) * NB],
            "w_qkv": w_qkv,
            "b_qkv": b_qkv,
            "w_fc": w_fc,
            "b_fc": b_fc,
        }
        for i in range(NCORES)
    ]
    kwargs = dict(_collect) if _collect else {}
    res = run_bass_kernel_spmd(nc, in_maps, core_ids=list(range(NCORES)), **kwargs)
    out = np.concatenate([res.results[i]["out"] for i in range(NCORES)], axis=0)
    if _collect is not None and isinstance(_collect, dict):
        _collect["result"] = res
    return out.astype(np.float32)


if __name__ == "__main__":
    xs = np.random.randn(B, S, D).astype(np.float32)
    lim = 1.0 / np.sqrt(D)
    rng = np.random.default_rng(0)
    wq = rng.uniform(-lim, lim, (D, 3 * D)).astype(np.float32)
    bq = rng.uniform(-lim, lim, (3 * D,)).astype(np.float32)
    wf = rng.uniform(-lim, lim, (D, D)).astype(np.float32)
    bf = rng.uniform(-lim, lim, (D,)).astype(np.float32)
    o = kernel(xs, wq, bq, wf, bf)
    print("out", o.shape, o.dtype)
